# revision 6
# baseline (speedup 1.0000x reference)
"""Trainium2 Bass kernel for nn_BitfieldLinear (vq_codebook).

y = x @ W^T + bias, where W[o,i] = basis[codes[o,i]] * scales[o].

Sharding: column-parallel over out_features across 8 cores. Each core:
  1. decodes its B^T = (basis[codes_shard] * scales)^T on-chip
     (GPSIMD ap_gather against a per-partition replicated 256-entry table,
      then PE transposes into [in_f, o_shard] layout),
  2. streams x tiles, PE-transposes them to put the contraction dim on
     partitions,
  3. runs float32r matmuls accumulating in PSUM, adds bias, writes its
     y shard. Host concatenates shards along the last dim.
"""

import os
import sys
from contextlib import ExitStack

sys.path.insert(0, "/opt/trn_rl_repo")

import numpy as np

import concourse.bass as bass
import concourse.mybir as mybir
import concourse.tile as tile
from concourse import bacc, library_config
from concourse.bass import ds, ts
from concourse.masks import make_identity

N_CORES = 8
IN_F = 4096
OUT_F = 4096
BASIS = 256
B_FULL = 4 * 2048

P = 128
F32 = mybir.dt.float32
F32R = mybir.dt.float32r
I32 = mybir.dt.int32
I16 = mybir.dt.int16

# rows of codes decoded per ap_gather call (one per 16-partition group)
ROWS_PER_CALL = 8


def build_program(b_rows: int, o_shard: int, mm_dtype=F32R):
    """Build the per-core bass program. Returns compiled nc."""
    nc = bacc.Bacc("TRN2", target_bir_lowering=False, debug=False)

    x_d = nc.dram_tensor("x", [b_rows, IN_F], F32, kind="ExternalInput").ap()
    codes_d = nc.dram_tensor("codes", [o_shard, IN_F], I32, kind="ExternalInput").ap()
    basis_d = nc.dram_tensor("basis", [1, BASIS], F32, kind="ExternalInput").ap()
    scales_d = nc.dram_tensor("scales", [o_shard, 1], F32, kind="ExternalInput").ap()
    bias_d = nc.dram_tensor("bias", [1, o_shard], F32, kind="ExternalInput").ap()
    y_d = nc.dram_tensor("y", [b_rows, o_shard], F32, kind="ExternalOutput").ap()

    NB = b_rows // P          # number of b tiles
    NK = IN_F // P            # number of K (j) tiles
    NOB = o_shard // P        # number of o blocks
    CALLS_PER_OB = P // ROWS_PER_CALL  # 16

    with tile.TileContext(nc) as tc, ExitStack() as ctx:
        # ---------------- constant pools ----------------
        const = ctx.enter_context(tc.tile_pool(name="const", bufs=1))
        identity = const.tile([P, P], F32)
        make_identity(nc, identity[:])

        basis_rep = const.tile([P, BASIS], F32)   # basis replicated per partition
        bias_bc = const.tile([P, o_shard], F32)   # bias replicated per partition
        scales_col = const.tile([P, NOB], F32)    # scales, one column per o-block
        ones_col = const.tile([1, P], F32)

        nc.vector.memset(ones_col[:], 1.0)

        # stage basis/bias rows, then broadcast across partitions with PE
        stage = ctx.enter_context(tc.tile_pool(name="stage", bufs=1))
        basis_row = stage.tile([1, BASIS], F32)
        bias_row = stage.tile([1, o_shard], F32)
        nc.sync.dma_start(out=basis_row[:], in_=basis_d[:, :])
        nc.sync.dma_start(out=bias_row[:], in_=bias_d[:, :])
        for ob in range(NOB):
            nc.sync.dma_start(
                out=scales_col[:, ds(ob, 1)], in_=scales_d[ts(ob, P), :]
            )

        ps_bc = ctx.enter_context(tc.tile_pool(name="ps_bc", bufs=1, space="PSUM"))
        bc_tile = ps_bc.tile([P, max(BASIS, o_shard)], F32)
        nc.tensor.matmul(
            bc_tile[:, :BASIS], ones_col[:], basis_row[:], start=True, stop=True
        )
        nc.vector.tensor_copy(basis_rep[:], bc_tile[:, :BASIS])
        nc.tensor.matmul(
            bc_tile[:, :o_shard], ones_col[:], bias_row[:], start=True, stop=True
        )
        nc.vector.tensor_copy(bias_bc[:], bc_tile[:, :o_shard])

        # B^T, decoded+scaled weights: [128 (j%P), NK (j//P), o_shard]
        bt_pool = ctx.enter_context(tc.tile_pool(name="bt", bufs=1))
        BT = bt_pool.tile([P, NK, o_shard], F32R)

        # ---------------- decode phase ----------------
        nc.gpsimd.load_library(library_config.ap_gather)

        with (
            tc.tile_pool(name="idx", bufs=4) as idx_pool,
            tc.tile_pool(name="gath", bufs=3) as g_pool,
            tc.tile_pool(name="dtile", bufs=2) as d_pool,
            tc.tile_pool(name="ps_tr", bufs=2, space="PSUM") as ps_tr,
        ):
            for ob in range(NOB):
                D = d_pool.tile([P, IN_F], F32)
                for t in range(CALLS_PER_OB):
                    # load codes rows (ob*P + t*8 + g) for g in 0..8, each row
                    # as a [16, IN_F//16] int32 block -> idx tile [128, IN_F//16]
                    idx32 = idx_pool.tile([P, IN_F // 16], I32, tag="idx32")
                    for g in range(ROWS_PER_CALL):
                        r = ob * P + t * ROWS_PER_CALL + g
                        nc.sync.dma_start(
                            out=idx32[ts(g, 16), :],
                            in_=codes_d[r : r + 1, :].rearrange(
                                "a (p s) -> (a p) s", p=16
                            ),
                        )
                    # keep the low int16 of each int32 (values < 256)
                    idx16 = idx_pool.tile([P, IN_F // 16], I16, tag="idx16")
                    nc.vector.tensor_copy(
                        idx16[:], idx32[:].bitcast(I16)[:, 0 :: 2]
                    )
                    # gather: each 16-partition group decodes one codes row,
                    # replicated across its 16 partitions
                    G = g_pool.tile([P, IN_F], F32)
                    nc.gpsimd.ap_gather(
                        G[:],
                        basis_rep[:],
                        idx16[:],
                        channels=P,
                        num_elems=BASIS,
                        d=1,
                        num_idxs=IN_F,
                    )
                    # pack: one representative partition per group -> D rows
                    nc.sync.dma_start(
                        out=D[ts(t, ROWS_PER_CALL), :],
                        in_=G[:].rearrange("(a b) f -> a b f", b=16)[:, 0, :],
                    )
                # scale rows by scales[o]
                nc.vector.tensor_scalar_mul(D[:], D[:], scales_col[:, ds(ob, 1)])
                # transpose D [128 o, IN_F j] into BT[:, :, ob*P:+P].
                # D's column j holds true i = 256*(j%16) + j//16 (the gather's
                # index order). For K-tile k the natural i-block [128k, 128k+128)
                # maps to columns j = 16*(i%256) + i//256 - a single stride-16
                # run starting at 2048*(k%2) + k//2.
                Dv = D[:].rearrange("p (s q) -> p q s", q=16)  # [128, 16, 256]
                for k4 in range(NK // 4):
                    pst = ps_tr.tile([P, 4 * P], F32)
                    for kk in range(4):
                        k = k4 * 4 + kk
                        c, r0 = k // 2, 128 * (k % 2)
                        nc.tensor.transpose(
                            pst[:, ts(kk, P)], Dv[:, c, ds(r0, P)], identity[:]
                        )
                    nc.vector.tensor_copy(
                        BT[:, ds(k4 * 4, 4), ts(ob, P)],
                        pst[:].rearrange("p (a b) -> p a b", a=4),
                    )

        # ---------------- main matmul loop ----------------
        x_pool = ctx.enter_context(tc.tile_pool(name="xin", bufs=2))
        xt_pool = ctx.enter_context(tc.tile_pool(name="xt", bufs=2))
        y_pool = ctx.enter_context(tc.tile_pool(name="yout", bufs=3))
        ps_xt = ctx.enter_context(tc.tile_pool(name="ps_xt", bufs=2, space="PSUM"))
        ps_mm = ctx.enter_context(tc.tile_pool(name="ps_mm", bufs=2, space="PSUM"))

        for bt in range(NB):
            xtile = x_pool.tile([P, IN_F], F32)
            nc.sync.dma_start(out=xtile[:], in_=x_d[ts(bt, P), :])
            xT = xt_pool.tile([P, NK * P], F32R)
            for k4 in range(NK // 4):
                pst = ps_xt.tile([P, 4 * P], F32)
                for kk in range(4):
                    k = k4 * 4 + kk
                    nc.tensor.transpose(
                        pst[:, ts(kk, P)], xtile[:, ts(k, P)], identity[:]
                    )
                # alternate copyback between ACT and DVE
                dst = xT[:, ts(k4, 4 * P)]
                if k4 % 2 == 0:
                    nc.scalar.copy(dst, pst[:])
                else:
                    nc.vector.tensor_copy(dst, pst[:])

            psum = ps_mm.tile([P, o_shard], F32)
            for k in range(NK):
                nc.tensor.matmul(
                    psum[:],
                    xT[:, ts(k, P)],
                    BT[:, k, :],
                    start=(k == 0),
                    stop=(k == NK - 1),
                )
            ytile = y_pool.tile([P, o_shard], F32)
            nc.vector.tensor_add(ytile[:], psum[:], bias_bc[:])
            nc.sync.dma_start(out=y_d[ts(bt, P), :], in_=ytile[:])

    nc.compile()
    return nc


_PROGRAM_CACHE = {}


def _get_program(b_rows: int, o_shard: int):
    key = (b_rows, o_shard)
    if key not in _PROGRAM_CACHE:
        _PROGRAM_CACHE[key] = build_program(b_rows, o_shard)
    return _PROGRAM_CACHE[key]


def kernel(x, codes, basis, scales, bias):
    from concourse.bass_utils import run_bass_kernel_spmd

    b_shape = x.shape[:-1]
    b_rows = int(np.prod(b_shape))
    out_f = codes.shape[0]
    o_shard = out_f // N_CORES

    xf = np.ascontiguousarray(x.reshape(b_rows, IN_F), dtype=np.float32)
    basis_r = np.ascontiguousarray(basis.reshape(1, BASIS), dtype=np.float32)

    in_maps = []
    for c in range(N_CORES):
        o0, o1 = c * o_shard, (c + 1) * o_shard
        in_maps.append(
            {
                "x": xf,
                "codes": np.ascontiguousarray(codes[o0:o1], dtype=np.int32),
                "basis": basis_r,
                "scales": np.ascontiguousarray(
                    scales[o0:o1].reshape(o_shard, 1), dtype=np.float32
                ),
                "bias": np.ascontiguousarray(
                    bias[o0:o1].reshape(1, o_shard), dtype=np.float32
                ),
            }
        )

    nc = _get_program(b_rows, o_shard)
    res = run_bass_kernel_spmd(nc, in_maps, list(range(N_CORES)))
    global LAST_RESULTS
    LAST_RESULTS = res
    y = np.concatenate([res.results[c]["y"] for c in range(N_CORES)], axis=1)
    return y.reshape(*b_shape, OUT_F).astype(np.float32)


LAST_RESULTS = None
